# revision 44
# baseline (speedup 1.0000x reference)
"""Trainium2 Bass kernel for RealVirtualAttention (masked segment-mean pool + HAN
semantic attention), SPMD across 8 NeuronCores.

v12 highlights (steady-state NEFF span ~190us/core, DMA-roofline-paced):
  - Data-parallel over graphs: 4096 graphs -> 128 blocks of GB=32; core i owns
    16 blocks, rank-matched into SPMD-uniform SLOTS (slot j's tile count =
    max over cores of the j-th largest block, by pair count).
  - Selector-sharing tile PAIRS: the host sorts each block's nodes by
    (graph, real|virt) category and splits every category run across the two
    tiles of a pair at identical positions (odd counts pad a zero row, ~1%
    inflation). One [128, 64] one-hot selector then serves TWO matmuls whose
    outputs go to different PE column-groups (psum rows 0:64 / 64:128), so
    LDWEIGHTS and matmuls overlap across col-groups (~63 ns/tile vs ~146).
  - One-hot selectors built in ONE batched DVE is_equal per slot
    (broadcast-AP compare of bf16 col ids vs iota), not per tile.
  - Main loop is paced by the xdat DMA stream (~330 GB/s sustained, bf16
    features, dual HWDGE queues, halved/quartered chunk transfers).
  - Slot epilogue (all-bf16 HAN head): ACT scale by 1/count -> means_bf; a
    [I64;I64] matmul transposes AND folds the even/odd psum halves; W1/tanh/q
    give per-slot scores accumulated into a running sum on DVE. Pipelined
    into the next slot's matmul stream.
  - Global beta: 4-byte AllGather of the local real-virt score diff; the
    combine is split so only `out = P1 + tanh(d/2B)*P2` (P1=(E0+E1)/2@means,
    P2=(E0-E1)/2@means, both computed DURING the collective) remains after
    it, followed by chunked output DMAs.
"""

import numpy as np
import ml_dtypes

import concourse.bacc as bacc
import concourse.bass as bass
import concourse.tile as tile
import concourse.mybir as mybir
from concourse.bass_utils import run_bass_kernel_spmd

F32 = mybir.dt.float32
BF16 = mybir.dt.bfloat16
NPBF16 = ml_dtypes.bfloat16
N_CORES = 8
B = 4096          # graphs
D = 150           # feature dim
A = 128           # attention hidden dim
GB = 32           # graphs per block
SW = 2 * GB       # selector width (real|virtual slots per block)
NBLK = 16         # blocks per core
VIRTUAL_Z = 100

_PROGRAM_CACHE: dict = {}
LAST_RESULTS = None  # BassKernelResults of the most recent run (for test.py)
LAST_NC = None       # compiled program of the most recent run (for test.py)
LAST_IN_MAPS = None  # per-core input maps of the most recent run (for test.py)


def _program_params(batch, z):
    """Per-slot tile counts (always even: tiles come in selector-sharing
    pairs). slot j holds each core's j-th largest block by PAIR count.

    A pair of tiles shares one one-hot selector; each category's nodes are
    split across the two tiles at the same positions (odd counts padded
    with a zero row), so a block needs sum_c ceil(n_c/2) positions."""
    virt = (z == VIRTUAL_Z)
    keyv = 2 * batch + virt
    cnt = np.bincount(keyv, minlength=2 * B).reshape(B, 2)
    pos_per_graph = (cnt + 1) // 2                       # [B, 2]
    pos_per_block = pos_per_graph.sum(axis=1).reshape(B // GB, GB).sum(axis=1)
    pairs = np.maximum((pos_per_block + 127) // 128, 1)  # [128 blocks]
    nbp = pairs.reshape(N_CORES, NBLK)
    s = -np.sort(-nbp, axis=1)
    Pj = s.max(axis=0)
    return tuple(int(2 * p) for p in Pj), 0


def _blob_offsets(T_list):
    """Column offsets of the packed f32 const blob [128, CBLOB].

    The per-node block-local col ids are stored as bf16 pairs packed into
    f32 words (width ceil(TT/2)) so the batched DVE compare reads bf16."""
    TTP = sum(T_list) // 2          # total selector-sharing pairs
    off = {}
    c = 0
    for name, w in [("ii", SW), ("col", (TTP + 1) // 2), ("scales", NBLK),
                    ("e0", GB), ("e1", GB), ("w1a", A), ("w1b", A),
                    ("b1", 1), ("q", 1)]:
        off[name] = c
        c += w
    return off, c


def _build_program(T_list, _unused: int = 0):
    key = ("v17", tuple(T_list))
    if key in _PROGRAM_CACHE:
        return _PROGRAM_CACHE[key]

    TT = sum(T_list)
    TTP = TT // 2
    TOTF = TT * D
    OFF, CBLOB = _blob_offsets(T_list)
    offx = np.concatenate([[0], np.cumsum([t * D for t in T_list])])
    offp = np.concatenate([[0], np.cumsum([t // 2 for t in T_list])])

    nc = bacc.Bacc("TRN2", target_bir_lowering=False, debug=False,
                   num_devices=N_CORES)
    xdat = nc.declare_dram_parameter("xdat", [128, TOTF], BF16, isOutput=False)
    blobp = nc.declare_dram_parameter("blob", [128, CBLOB], F32, isOutput=False)
    iotap = nc.declare_dram_parameter("iota", [128, SW], BF16, isOutput=False)
    resp = nc.declare_dram_parameter("res", [GB, NBLK, D], F32, isOutput=True)
    resp2 = resp.reshape([GB, NBLK * D]) if hasattr(resp, "reshape") else resp

    with tile.TileContext(nc) as tc:
        with tc.tile_pool(name="const", bufs=1) as cpool, \
             tc.tile_pool(name="chunks", bufs=5) as chpool, \
             tc.tile_pool(name="oh", bufs=8) as ohpool, \
             tc.tile_pool(name="small", bufs=1) as spool, \
             tc.tile_pool(name="xt", bufs=2) as xtpool, \
             tc.tile_pool(name="pm", bufs=2, space="PSUM") as pm, \
             tc.tile_pool(name="ptp", bufs=1, space="PSUM") as ptp, \
             tc.tile_pool(name="ph", bufs=1, space="PSUM") as ph, \
             tc.tile_pool(name="ps", bufs=1, space="PSUM") as ps, \
             tc.tile_pool(name="pbb", bufs=1, space="PSUM") as pbbp, \
             tc.tile_pool(name="pout", bufs=2, space="PSUM") as pout, \
             tc.tile_pool(name="dram", bufs=1, space="DRAM") as dpool:

            # --- constants (one blob DMA + iota) ---
            blob_t = cpool.tile([128, CBLOB], F32, tag="blob")
            nc.scalar.dma_start(blob_t[:], blobp[:])
            iota_t = cpool.tile([128, SW], BF16, tag="iota")
            nc.scalar.dma_start(iota_t[:], iotap[:])

            def bs(name, w, p=128):
                c = OFF[name]
                return blob_t[0:p, c:c + w]

            ii_t = bs("ii", SW)                 # [128, 64] = [I64; I64]
            col_t = bs("col", (TTP + 1) // 2).bitcast(BF16)  # [128, ~TTP] bf16
            scales_t = bs("scales", NBLK)       # [128, NBLK], rows c==64+c
            e0_t = bs("e0", GB)                 # [128, GB] (ED, duplicated)
            e1_t = bs("e1", GB)                 # [128, GB] (E1, duplicated)
            w1a_t = bs("w1a", A)
            w1b_t = blob_t[0:D - 128, OFF["w1b"]:OFF["w1b"] + A]
            b1_t = bs("b1", 1)
            q_t = bs("q", 1)

            means_bf = cpool.tile([128, NBLK * D], BF16, tag="meansbf")
            srun = cpool.tile([1, SW], F32, tag="srun")
            nc.vector.memset(srun[:], 0.0)
            osb_all = cpool.tile([GB, NBLK * D], F32, tag="osb")
            p1sb = cpool.tile([GB, NBLK * D], F32, tag="p1sb")
            p2sb = cpool.tile([GB, NBLK * D], F32, tag="p2sb")

            eq = mybir.AluOpType.is_equal
            mult = mybir.AluOpType.mult

            # one-time bf16 mirrors of the small HAN-head constants
            ii_bf = cpool.tile([128, SW], BF16, tag="iibf")
            nc.scalar.copy(ii_bf[:], ii_t)
            w1a_bf = cpool.tile([128, A], BF16, tag="w1abf")
            nc.scalar.copy(w1a_bf[:], w1a_t)
            w1b_bf = cpool.tile([D - 128, A], BF16, tag="w1bbf")
            nc.scalar.copy(w1b_bf[:], w1b_t)
            q_bf = cpool.tile([128, 1], BF16, tag="qbf")
            nc.scalar.copy(q_bf[:], q_t)
            ed_bf = cpool.tile([128, GB], BF16, tag="edbf")
            nc.scalar.copy(ed_bf[:], e0_t)
            e1_bf = cpool.tile([128, GB], BF16, tag="e1bf")
            nc.scalar.copy(e1_bf[:], e1_t)

            psum_tiles = [None] * NBLK

            def epilogue(j):
                """means scale + HAN head for slot j (ACT/PE only; no DVE).

                psum rows 0:64 / 64:128 hold the even-/odd-tile partial
                sums; the [I;I] matmul transposes AND folds the halves."""
                msl = means_bf[:, j * D:(j + 1) * D]
                nc.scalar.mul(msl, psum_tiles[j][:],
                              scales_t[:, j:j + 1])
                tp = ptp.tile([128, 128], F32, tag="tp")
                nc.tensor.matmul(tp[:, 0:SW], msl[:, 0:128], ii_bf[:],
                                 start=True, stop=True)
                nc.tensor.matmul(tp[0:D - 128, SW:2 * SW], msl[:, 128:D],
                                 ii_bf[:], start=True, stop=True)
                xt = xtpool.tile([128, 128], BF16, tag="xt")
                nc.scalar.copy(xt[:], tp[:])
                ph_t = ph.tile([128, SW], F32, tag="h")
                nc.tensor.matmul(ph_t[:], w1a_bf[:], xt[:, 0:SW],
                                 start=True, stop=False)
                nc.tensor.matmul(ph_t[:], w1b_bf[:], xt[0:D - 128, SW:2 * SW],
                                 start=False, stop=True)
                ht = xtpool.tile([128, SW], BF16, tag="ht")
                nc.scalar.activation(ht[:], ph_t[:],
                                     mybir.ActivationFunctionType.Tanh,
                                     bias=b1_t)
                ps_t = ps.tile([1, SW], F32, tag="s")
                nc.tensor.matmul(ps_t[:], q_bf[:], ht[:], start=True, stop=True)
                nc.vector.tensor_add(srun[:], srun[:], ps_t[:])

            # --- main streaming loop: masked segment sums per slot ---
            for j in range(NBLK):
                Tj = T_list[j]
                Tp = Tj // 2
                psum_tiles[j] = pm.tile([128, D], F32, tag="pmeans",
                                        name="pmeans")
                chunk = chpool.tile([128, Tj * D], BF16, tag="chunk")
                H = (Tj // 2) * D
                # alternate the two HWDGE rings (sync=SP, scalar=ACT) so the
                # SDMA engines round-robin two queues
                eng = nc.sync if j % 2 == 0 else nc.scalar
                eng2 = nc.scalar if j % 2 == 0 else nc.sync
                if j == 0:
                    # split the very first transfer (eighth, then the rest)
                    # so the PE can start as soon as ~1/8 of slot 0 lands
                    Q8 = max((Tp // 4) & ~1, 2) * D
                    Q = (Tp // 2) * D
                    eng.dma_start(chunk[:, 0:Q8],
                                  xdat[:, offx[j]:offx[j] + Q8])
                    eng.dma_start(chunk[:, Q8:Q],
                                  xdat[:, offx[j] + Q8:offx[j] + Q])
                    eng.dma_start(chunk[:, Q:H],
                                  xdat[:, offx[j] + Q:offx[j] + H])
                else:
                    eng.dma_start(chunk[:, 0:H],
                                  xdat[:, offx[j]:offx[j] + H])
                if j == NBLK - 1:
                    # quarter-split the stream's final piece so the PE tail
                    # after the last byte is only ~a quarter slot of matmuls
                    M = H + ((Tj - Tp) // 2) * D
                    eng2.dma_start(chunk[:, H:M],
                                   xdat[:, offx[j] + H:offx[j] + M])
                    eng2.dma_start(chunk[:, M:Tj * D],
                                   xdat[:, offx[j] + M:offx[j + 1]])
                else:
                    eng2.dma_start(chunk[:, H:Tj * D],
                                   xdat[:, offx[j] + H:offx[j + 1]])
                # one batched compare builds the Tp pair selectors:
                # oh[p, i, c] = (iota[c] == col[p, offp_j + i])
                oh = ohpool.tile([128, Tp * SW], BF16, tag="oh")
                oh3 = oh[:].rearrange("p (t c) -> p t c", c=SW)
                splits = [0, min(8, Tp), Tp] if j == 0 else [0, Tp]
                for a, b in zip(splits, splits[1:]):
                    if a == b:
                        continue
                    nc.vector.tensor_tensor(
                        out=oh3[:, a:b, :],
                        in0=iota_t[:][:, None, :].broadcast_to([128, b - a, SW]),
                        in1=col_t[:, offp[j] + a:offp[j] + b][:, :, None]
                            .broadcast_to([128, b - a, SW]),
                        op=eq)
                # pair i: tiles 2i/2i+1 share selector i; the even tile
                # accumulates into psum rows 0:64 (PE col-group 0), the odd
                # into rows 64:128 (col-group 1) — one LDWEIGHTS feeds two
                # matmuls in different col-groups, so loads overlap compute.
                for i in range(Tp):
                    if i == 8 and j > 0:
                        epilogue(j - 1)
                    lhs = oh[:, i * SW:(i + 1) * SW]
                    nc.tensor.matmul(psum_tiles[j][0:SW, :], lhs,
                                     chunk[:, (2 * i) * D:(2 * i + 1) * D],
                                     start=(i == 0), stop=(i == Tp - 1))
                    nc.tensor.matmul(psum_tiles[j][SW:128, :], lhs,
                                     chunk[:, (2 * i + 1) * D:(2 * i + 2) * D],
                                     start=(i == 0), stop=(i == Tp - 1))
            epilogue(NBLK - 1)

            # --- global beta via 4-byte AllGather of the local score diff ---
            d2 = spool.tile([1, GB], F32, tag="d2")
            nc.vector.tensor_tensor(out=d2[:], in0=srun[0:1, 0:GB],
                                    in1=srun[0:1, GB:SW],
                                    op=mybir.AluOpType.subtract)
            s2 = spool.tile([1, 1], F32, tag="s2")
            nc.vector.reduce_sum(out=s2[:], in_=d2[:],
                                 axis=mybir.AxisListType.X)
            cc_in = dpool.tile([1, 1], F32)
            cc_out = dpool.tile([N_CORES, 1], F32)
            ones8 = spool.tile([N_CORES, 1], F32, tag="ones8")
            nc.vector.memset(ones8[:], 1.0)
            nc.scalar.dma_start(cc_in[:], s2[:])
            # AllGather (W-1 ring hops) beats AllReduce (2(W-1)) for 4 bytes;
            # the 8-way sum is one tiny matmul.
            nc.gpsimd.collective_compute(
                "AllGather", mybir.AluOpType.bypass,
                replica_groups=[list(range(N_CORES))],
                ins=[cc_in.opt()], outs=[cc_out.opt()])

            # --- beta-independent combine halves, overlapped with the CC:
            #     P1 = (E1 + ED/2).T @ means  (the beta=1/2 midpoint; DMAd
            #     to the output now), P2 = (ED/2).T @ means (half-diff)
            CW = 480
            NC5 = (NBLK * D + CW - 1) // CW
            HALF = 3 * CW
            for i in range(NC5):
                lo = i * CW
                hi = min(NBLK * D, lo + CW)
                po2 = pout.tile([GB, CW], F32, tag="po")
                nc.tensor.matmul(po2[0:GB, 0:hi - lo], ed_bf[:],
                                 means_bf[:, lo:hi], start=True, stop=True)
                nc.scalar.copy(p2sb[:, lo:hi], po2[0:GB, 0:hi - lo])
                po1 = pout.tile([GB, CW], F32, tag="po")
                nc.tensor.matmul(po1[0:GB, 0:hi - lo], e1_bf[:],
                                 means_bf[:, lo:hi], start=True, stop=True)
                nc.scalar.copy(p1sb[:, lo:hi], po1[0:GB, 0:hi - lo])

            # --- beta tail: t = tanh(global_diff / 2B)  (sigmoid == affine
            #     of tanh; tanh's ACT table is already loaded) ---
            sg8 = spool.tile([N_CORES, 1], F32, tag="sg8")
            nc.scalar.dma_start(sg8[:], cc_out[:])
            psg = pbbp.tile([1, 1], F32, tag="bb", name="psg")
            nc.tensor.matmul(psg[:], ones8[:], sg8[:], start=True, stop=True)
            b0 = spool.tile([1, 1], F32, tag="b0")
            nc.scalar.activation(b0[:], psg[:],
                                 mybir.ActivationFunctionType.Tanh,
                                 scale=0.5 / B)
            ones_t = spool.tile([1, GB], F32, tag="ones")
            nc.vector.memset(ones_t[:], 1.0)
            pbb = pbbp.tile([GB, 1], F32, tag="bb")
            nc.tensor.matmul(pbb[:], ones_t[:], b0[:], start=True, stop=True)
            beta_bc8 = spool.tile([GB, 1], F32, tag="bbc")
            nc.scalar.copy(beta_bc8[:], pbb[:])
            beta_bc = beta_bc8[:, 0:1]

            # --- post-CC fold: out = P1 + t * P2, chunked output DMA.
            #     beta-multiplies on ACT so the DVE adds pipeline behind them
            for i in range(NC5):
                lo = i * CW
                hi = min(NBLK * D, lo + CW)
                nc.scalar.mul(osb_all[:, lo:hi], p2sb[:, lo:hi], beta_bc)
                nc.vector.tensor_add(osb_all[:, lo:hi], osb_all[:, lo:hi],
                                     p1sb[:, lo:hi])
                if hi == HALF:
                    nc.scalar.dma_start(resp2[:, 0:HALF], osb_all[:, 0:HALF])
            nc.scalar.dma_start(resp2[:, HALF:NBLK * D],
                                osb_all[:, HALF:NBLK * D])

    nc.compile()
    _PROGRAM_CACHE[key] = nc
    return nc


def kernel(out, z, batch, W1, b1, q, num_graphs):
    global LAST_RESULTS, LAST_NC, LAST_IN_MAPS
    out = np.ascontiguousarray(np.asarray(out, dtype=np.float32))
    z = np.asarray(z).astype(np.int64)
    batch = np.asarray(batch).astype(np.int64)
    W1 = np.asarray(W1, dtype=np.float32)
    b1 = np.asarray(b1, dtype=np.float32)
    q = np.asarray(q, dtype=np.float32)
    assert int(num_graphs) == B
    assert out.shape[1] == D and W1.shape == (D, A)

    cuts = np.searchsorted(batch, np.arange(0, B + 1, GB))
    T_list, _ = _program_params(batch, z)
    TT = sum(T_list)
    TTP = TT // 2
    OFF, CBLOB = _blob_offsets(T_list)
    offx = np.concatenate([[0], np.cumsum([t * D for t in T_list])])
    offt = np.concatenate([[0], np.cumsum(T_list)])
    offp = np.concatenate([[0], np.cumsum([t // 2 for t in T_list])])

    virt = (z == VIRTUAL_Z)
    keyv = 2 * batch + virt
    icnt = np.bincount(keyv, minlength=2 * B).reshape(B, 2)
    rcnt = 1.0 / np.maximum(icnt.astype(np.float32), 1.0)   # [B, 2]
    pos_per_block = ((icnt + 1) // 2).sum(axis=1).reshape(B // GB, GB)\
        .sum(axis=1)
    pairs_blk = np.maximum((pos_per_block + 127) // 128, 1)\
        .reshape(N_CORES, NBLK)

    xb = out.astype(NPBF16)
    colf = ((batch % GB) + GB * virt).astype(np.int32)      # block-local col id

    iota = np.tile(np.arange(SW, dtype=NPBF16), (128, 1))

    blob_common = np.zeros((128, CBLOB), dtype=np.float32)
    # [I64; I64] — folds the even/odd psum halves during the transpose matmul
    blob_common[0:SW, OFF["ii"]:OFF["ii"] + SW] = np.eye(SW)
    blob_common[SW:128, OFF["ii"]:OFF["ii"] + SW] = np.eye(SW)
    # e0 slot holds ED/2, e1 slot holds E1 + ED/2 = (E0+E1)/2, so the
    # combine is out = e1@m + tanh(d/2B)*(e0@m); duplicated on rows 64:128
    # so the combine matmul also folds the even/odd psum halves.
    for r0 in (0, SW):
        blob_common[r0 + 0:r0 + GB, OFF["e0"]:OFF["e0"] + GB] = 0.5 * np.eye(GB)
        blob_common[r0 + GB:r0 + SW, OFF["e0"]:OFF["e0"] + GB] = -0.5 * np.eye(GB)
        blob_common[r0 + 0:r0 + GB, OFF["e1"]:OFF["e1"] + GB] = 0.5 * np.eye(GB)
        blob_common[r0 + GB:r0 + SW, OFF["e1"]:OFF["e1"] + GB] = 0.5 * np.eye(GB)
    blob_common[:, OFF["w1a"]:OFF["w1a"] + A] = W1[:128]
    blob_common[0:D - 128, OFF["w1b"]:OFF["w1b"] + A] = W1[128:]
    blob_common[:, OFF["b1"]] = b1
    blob_common[:, OFF["q"]] = q.reshape(A)

    in_maps = []
    orders = []
    for core in range(N_CORES):
        order = np.argsort(-pairs_blk[core], kind="stable")  # slot j <- block
        orders.append(order)
        arr = np.zeros((TT * 128, D), dtype=NPBF16)
        colv = np.full(TTP * 128, -1, dtype=np.int32)        # [pair, pos]
        blob = blob_common.copy()
        g0 = core * NBLK * GB
        for j in range(NBLK):
            k = NBLK * core + int(order[j])
            lo, hi = int(cuts[k]), int(cuts[k + 1])
            # nodes sorted by category; category runs split across tile
            # pairs at matching positions (odd remainders -> zero-row pad)
            sidx = np.argsort(colf[lo:hi], kind="stable") + lo
            scats = colf[sidx]
            bnd = np.flatnonzero(np.diff(scats)) + 1
            starts = np.concatenate([[0], bnd])
            ends = np.concatenate([bnd, [len(scats)]])
            tbase = offt[j] * 128
            cbase = offp[j] * 128
            p_pair, cur = 0, 0
            for s, e in zip(starts, ends):
                c = int(scats[s])
                r = e - s
                n = 0
                while n < r:
                    if cur == 128:
                        p_pair += 1
                        cur = 0
                    space = 128 - cur
                    rem = r - n
                    if rem >= 2 * space:
                        q, take = space, 2 * space
                    else:
                        q, take = (rem + 1) // 2, rem
                    l0 = sidx[s + n:s + n + q]
                    l1 = sidx[s + n + q:s + n + take]
                    r0 = tbase + (2 * p_pair) * 128 + cur
                    arr[r0:r0 + q] = xb[l0]
                    r1 = tbase + (2 * p_pair + 1) * 128 + cur
                    arr[r1:r1 + len(l1)] = xb[l1]
                    colv[cbase + p_pair * 128 + cur:
                         cbase + p_pair * 128 + cur + q] = c
                    n += take
                    cur += q
            gids = g0 + int(order[j]) * GB + np.arange(GB)
            for r0 in (0, SW):
                blob[r0 + 0:r0 + GB, OFF["scales"] + j] = rcnt[gids, 0]
                blob[r0 + GB:r0 + SW, OFF["scales"] + j] = rcnt[gids, 1]
        # per-slot tile-major layout: [128, sum_j T_j*D]
        xarr = np.empty((128, TT * D), dtype=NPBF16)
        for j in range(NBLK):
            seg = arr[offt[j] * 128:offt[j + 1] * 128]      # [T_j*128, D]
            Tj = T_list[j]
            xarr[:, offx[j]:offx[j + 1]] = (
                seg.reshape(Tj, 128, D).transpose(1, 0, 2).reshape(128, Tj * D))
        WC = (TTP + 1) // 2
        col16 = np.full((128, 2 * WC), -1.0, dtype=NPBF16)
        col16[:, 0:TTP] = colv.reshape(TTP, 128).T.astype(NPBF16)
        blob[:, OFF["col"]:OFF["col"] + WC] = col16.view(np.float32)
        in_maps.append({"xdat": xarr, "blob": blob, "iota": iota})

    nc = _build_program(T_list)
    LAST_NC, LAST_IN_MAPS = nc, in_maps
    res = run_bass_kernel_spmd(nc, in_maps, core_ids=list(range(N_CORES)))
    LAST_RESULTS = res
    parts = []
    for i in range(N_CORES):
        slots = res.results[i]["res"].transpose(1, 0, 2)    # [NBLK slots, GB, D]
        blocks = np.empty_like(slots)
        blocks[orders[i]] = slots                           # un-permute
        parts.append(blocks.reshape(NBLK * GB, D))
    return np.ascontiguousarray(np.concatenate(parts, axis=0), dtype=np.float32)

